# revision 24
# baseline (speedup 1.0000x reference)
"""Multi-head causal attention (RoPE) on 8 TRN2 NeuronCores.

Sharding: tensor-parallel over heads. Each core computes 2 of the 16 heads:
column-parallel q/k/v projections, local attention, then a per-batch-row
AllToAll of the transposed attention outputs and a token-parallel o-proj
(each core produces the full 1024-wide output for 128 tokens per row).

Layout strategy: activations live transposed on-chip ([dim, token]) so every
matmul contracts over the partition axis with no transposes of x. Scores are
computed transposed ([tk, tq]); softmax has no max-subtraction (logits are
O(1) for this input distribution) and its denominator is produced by a
64-wide ones block appended to V in the PV matmul; normalization is a single
tensor-tensor divide per (b, head, tq-half) writing bf16 aoT directly.
RoPE uses the interleaved-pair identity q' = q*C + swap(q)*S', with the pair
swap done by the DVE stream-shuffle.

o-proj is token-stationary: after the per-row AllToAll each core holds all
1024 attention dims for its 128 tokens of that row; the 128-token tile is the
matmul stationary operand and Wo.T streams as the moving operand (N=512).

Schedule: attention(b) is emitted with next-row QKV projection units and the
o-proj of row b-2 INTERLEAVED between its score/PV groups, so the in-order
PE queue always has ready matmuls behind an exp-gated attention group (keeps
the HAM clock-gate warm and hides dependency stalls). The per-row AllToAll
is issued right after row b's last normalize; o-proj lags two rows so a
late-started peer's collective never head-of-line-blocks the PE queue.
DMA priority: x row 0 (4 chunked DMAs) first, then rows 1-3 (one DMA each),
then Wo on the same FIFO ring so it never competes with the x load.
"""

import sys

for _p in ("/opt/trn_rl_repo",):
    if _p not in sys.path:
        sys.path.insert(0, _p)

import contextlib

import numpy as np
import ml_dtypes

import concourse.bass as bass
import concourse.mybir as mybir
import concourse.tile as tile
from concourse import bacc
from concourse.bass_utils import run_bass_kernel_spmd
from concourse.masks import make_identity

# Problem constants (nn_MultiHeadAttention: x [4,1024,1024], 16 heads)
B, T, D = 4, 1024, 1024
H, DH = 16, 64
NCORES = 8
HPC = H // NCORES          # heads per core = 2
DPC = HPC * DH             # head-dims per core = 128
BT = B * T                 # 4096 tokens
CT = D // 128              # 8 contraction tiles of 128
TPB = T // 128             # 8 key/query 128-tiles per batch row
ROPE_BASE = 10000.0

F32 = mybir.dt.float32
BF16 = mybir.dt.bfloat16
AF = mybir.ActivationFunctionType
ALU = mybir.AluOpType

SWAP_MASK = [i ^ 1 for i in range(32)]  # pair swap within each 32-partition group

_compiled = {}


def _build_nc():
    nc = bacc.Bacc(None, target_bir_lowering=False, debug=False)

    xT = nc.declare_dram_parameter("xT", [D, BT], BF16, isOutput=False)
    # qkv weights prepacked on host to [128, CT*128] (SBUF layout, single DMA)
    wq = nc.declare_dram_parameter("wq", [128, CT * DPC], BF16, isOutput=False)
    wk = nc.declare_dram_parameter("wk", [128, CT * DPC], BF16, isOutput=False)
    wv = nc.declare_dram_parameter("wv", [128, CT * DPC], BF16, isOutput=False)
    # wo packed [128, CT*D]: block ct = Wo.T rows [128ct:128(ct+1)] (all 1024 cols)
    wo = nc.declare_dram_parameter("wo", [128, CT * D], BF16, isOutput=False)
    cosb = nc.declare_dram_parameter("cosb", [DPC, T], BF16, isOutput=False)
    sinb = nc.declare_dram_parameter("sinb", [DPC, T], BF16, isOutput=False)
    triu = nc.declare_dram_parameter("triu", [128, 128], BF16, isOutput=False)
    # output [tokens, e]: rows [128b:128(b+1)] = batch row b, my 128 tokens
    yO = nc.declare_dram_parameter("yO", [B * 128, D], F32, isOutput=True)

    with tile.TileContext(nc) as tc:
        with contextlib.ExitStack() as ctx:
            dram = ctx.enter_context(tc.tile_pool(name="dram", bufs=1, space="DRAM"))
            # per-batch-row AllToAll bounce buffers
            ag_in = [dram.tile([D, 128], BF16, name=f"agin{b}") for b in range(B)]
            ag_out = [dram.tile([D, 128], BF16, name=f"agout{b}") for b in range(B)]

            consts = ctx.enter_context(tc.tile_pool(name="consts", bufs=1))

            # small weights on the scalar (Activation) DMA ring: loaded in
            # parallel with row 0's x chunks on the sync ring
            cos_sb = consts.tile([DPC, T], BF16)
            sin_sb = consts.tile([DPC, T], BF16)
            triu_sb = consts.tile([128, 128], BF16)
            w_sbs = {}
            for wname, w_dr in (("wq", wq), ("wk", wk), ("wv", wv)):
                w_sb = consts.tile(list(w_dr.shape), BF16, name=f"{wname}_sb")
                nc.scalar.dma_start(w_sb[:], w_dr[:])
                w_sbs[wname] = w_sb
            nc.scalar.dma_start(cos_sb[:], cosb[:])
            nc.scalar.dma_start(sin_sb[:], sinb[:])
            nc.scalar.dma_start(triu_sb[:], triu[:])
            wq_sb, wk_sb, wv_sb = (w_sbs[n] for n in ("wq", "wk", "wv"))
            wo_sb = consts.tile([128, CT * D], BF16, name="wo_sb")

            xpool = ctx.enter_context(tc.tile_pool(name="xTp", bufs=1))
            # row 0 in 4 chunked DMAs (2 ct-tiles each) so the first QKV
            # matmuls start after ~0.5 MB instead of 2 MB; rows 1-3 as one
            # DMA each (sequencer issue is ~0.7us per dma_start)
            xrows = []
            xrow0 = xpool.tile([128, CT, T], BF16, tag="xr0", name="xrow0")
            for c4 in range(4):
                nc.sync.dma_start(
                    xrow0[:, 2 * c4:2 * c4 + 2, :],
                    xT[2 * c4 * 128:(2 * c4 + 2) * 128, 0:T]
                    .rearrange("(ct p) t -> p ct t", p=128))
            xrows.append(xrow0)
            for b in range(1, B):
                xrow = xpool.tile([128, CT, T], BF16, tag=f"xr{b}",
                                  name=f"xrow{b}")
                nc.sync.dma_start(
                    xrow[:],
                    xT[:, b * T:(b + 1) * T].rearrange("(ct p) t -> p ct t",
                                                       p=128))
                xrows.append(xrow)
            # wo (2 MB, needed only by o-proj) goes on the sync ring strictly
            # AFTER the x rows: the ring is FIFO, so it never steals x
            # bandwidth during the startup-critical phase
            nc.sync.dma_start(wo_sb[:], wo[:])

            ident = consts.tile([128, 128], BF16)
            make_identity(nc, ident[:])

            pers = ctx.enter_context(tc.tile_pool(name="pers", bufs=1))
            qT_sb = pers.tile([128, BT], BF16)
            kT_sb = pers.tile([128, BT], BF16)
            aoT_sb = pers.tile([128, BT], BF16)
            # persistent [ones | v_h0 | ones | v_h1] PV lhsT tiles; the ones
            # columns are set once and survive across rows
            v_tiles = [pers.tile([128, 256], BF16, name=f"v{kt}")
                       for kt in range(TPB)]
            for kt in range(TPB):
                nc.gpsimd.memset(v_tiles[kt][:, 0:64], 1.0)
                nc.gpsimd.memset(v_tiles[kt][:, 128:192], 1.0)

            ppool = ctx.enter_context(
                tc.tile_pool(name="proj_psum", bufs=2, space="PSUM"))
            rtp = ctx.enter_context(tc.tile_pool(name="rope_tmp", bufs=2))
            vtmp = ctx.enter_context(tc.tile_pool(name="vtmp", bufs=2))
            epool = ctx.enter_context(tc.tile_pool(name="E", bufs=1))
            spsum = ctx.enter_context(
                tc.tile_pool(name="s_psum", bufs=2, space="PSUM"))
            opsum = ctx.enter_context(
                tc.tile_pool(name="o_psum", bufs=2, space="PSUM"))
            aof_pool = ctx.enter_context(tc.tile_pool(name="aof", bufs=2))
            yout = ctx.enter_context(tc.tile_pool(name="yout", bufs=2))

            scale = float(DH) ** -0.5
            vts_store = {b: {} for b in range(B)}

            def qkv_units(b):
                """Per-(chunk, projection) closures: 8-MM groups + RoPE."""
                units = []
                for ci in range(2):
                    ch = 2 * b + ci
                    sl = slice(ch * 512, ch * 512 + 512)
                    tsl = slice(ci * 512, ci * 512 + 512)
                    xsl = slice(ci * 512, ci * 512 + 512)

                    def mk_v(b=b, ci=ci, ch=ch, xsl=xsl):
                        pv = ppool.tile([128, 512], F32, tag="proj",
                                        name=f"pv{ch}")
                        for ct in range(CT):
                            nc.tensor.matmul(
                                pv[:], wv_sb[:, ct * DPC:(ct + 1) * DPC],
                                xrows[b][:, ct, xsl],
                                start=(ct == 0), stop=(ct == CT - 1))
                        vt = vtmp.tile([128, 512], BF16, tag="vt",
                                       name=f"vt{ch}")
                        nc.vector.tensor_copy(vt[:], pv[:])
                        vts_store[b][ci] = vt
                    units.append(mk_v)

                    for wsb, dst, pname in ((wq_sb, qT_sb, "pq"),
                                            (wk_sb, kT_sb, "pk")):
                        def mk_qk(b=b, ch=ch, sl=sl, tsl=tsl, xsl=xsl,
                                  wsb=wsb, dst=dst, pname=pname):
                            pp = ppool.tile([128, 512], F32, tag="proj",
                                            name=f"{pname}{ch}")
                            for ct in range(CT):
                                nc.tensor.matmul(
                                    pp[:], wsb[:, ct * DPC:(ct + 1) * DPC],
                                    xrows[b][:, ct, xsl],
                                    start=(ct == 0), stop=(ct == CT - 1))
                            # stream_shuffle needs an SBUF source; the
                            # cos-mult reads the projection PSUM directly
                            qraw = rtp.tile([128, 512], BF16, tag="qraw",
                                            name=f"qraw{pname}{ch}")
                            nc.scalar.copy(qraw[:], pp[:])
                            sw = rtp.tile([128, 512], BF16, tag="sw",
                                          name=f"sw{pname}{ch}")
                            m1 = rtp.tile([128, 512], BF16, tag="m1",
                                          name=f"m1{pname}{ch}")
                            m2 = rtp.tile([128, 512], BF16, tag="m2",
                                          name=f"m2{pname}{ch}")
                            nc.vector.stream_shuffle(sw[:], qraw[:], SWAP_MASK)
                            nc.vector.tensor_tensor(m1[:], pp[:],
                                                    cos_sb[:, tsl], ALU.mult)
                            nc.vector.tensor_tensor(m2[:], sw[:],
                                                    sin_sb[:, tsl], ALU.mult)
                            nc.vector.tensor_tensor(dst[:, sl], m1[:], m2[:],
                                                    ALU.add)
                        units.append(mk_qk)
                return units

            def transpose_unit(b, kt):
                """[d, t] -> v_tiles[kt] [ones|v_h0|ones|v_h1] columns."""
                vts = vts_store[b]
                pt = ppool.tile([128, 128], BF16, tag="proj", name=f"pt{b}_{kt}")
                nc.tensor.transpose(pt[:], vts[kt // 4][:, (kt % 4) * 128:
                                                        (kt % 4) * 128 + 128],
                                    ident[:])
                nc.scalar.copy(v_tiles[kt][:, 64:128], pt[:, 0:64])
                nc.vector.tensor_copy(v_tiles[kt][:, 192:256], pt[:, 64:128])

            def attention(b, fillers):
                """Scores+exp+PV+normalize for row b; fillers (next-row QKV
                units, o-proj of row b-2) are emitted between groups so the
                PE queue always has ready work behind exp-gated groups."""
                b0 = b * T
                nfill = len(fillers)
                emitted = 0
                point = 0

                def fill_point():
                    nonlocal emitted, point
                    point += 1
                    target = (point * nfill) // 12
                    while emitted < target:
                        fillers[emitted]()
                        emitted += 1

                e_tiles = {}
                for h in range(HPC):
                    for kt in range(TPB):
                        e_tiles[(h, kt)] = epool.tile(
                            [128, T], BF16, tag=f"e{h}_{kt}", name=f"e{b}_{h}_{kt}")
                for kt in range(TPB):
                    transpose_unit(b, kt)
                    lo = kt * 128
                    for h in range(HPC):
                        hsl = slice(h * 64, (h + 1) * 64)
                        ps = spsum.tile([128, T], F32, tag="s", name=f"s{b}_{h}_{kt}")
                        # bank-aligned score matmuls over the valid range only
                        if lo < 512:
                            nc.tensor.matmul(ps[:, lo:512],
                                             kT_sb[hsl, b0 + lo:b0 + lo + 128],
                                             qT_sb[hsl, b0 + lo:b0 + 512],
                                             start=True, stop=True)
                        nc.tensor.matmul(ps[:, max(lo, 512):T],
                                         kT_sb[hsl, b0 + lo:b0 + lo + 128],
                                         qT_sb[hsl, b0 + max(lo, 512):b0 + T],
                                         start=True, stop=True)
                        nc.scalar.activation(e_tiles[(h, kt)][:, lo:T], ps[:, lo:T],
                                             AF.Exp, scale=scale)
                        nc.vector.tensor_tensor(
                            e_tiles[(h, kt)][:, lo:lo + 128],
                            e_tiles[(h, kt)][:, lo:lo + 128],
                            triu_sb[:], ALU.mult)
                    fill_point()
                for h in range(HPC):
                    # lhsT = [ones | v_h]: PSUM rows 0:64 = denom (at base
                    # partition 0, which the custom-DVE reciprocal requires),
                    # rows 64:128 = PV.
                    for half in range(2):
                        c0 = half * 512
                        po = opsum.tile([128, 512], F32, tag="po",
                                        name=f"po{b}_{h}_{half}")
                        nkt = TPB if half else 4
                        for kt in range(nkt):
                            lo = max(kt * 128 - c0, 0)
                            nc.tensor.matmul(
                                po[:, lo:512],
                                v_tiles[kt][:, h * 128:h * 128 + 128],
                                e_tiles[(h, kt)][:, c0 + lo:c0 + 512],
                                start=(kt == 0), stop=(kt == nkt - 1))
                        den = rtp.tile([64, 512], F32, tag="den",
                                      name=f"den{b}_{h}_{half}")
                        nc.vector.reciprocal_approx_fast(den[:], po[0:64, :])
                        nc.vector.tensor_tensor(
                            aoT_sb[h * 64:(h + 1) * 64, b0 + c0:b0 + c0 + 512],
                            po[64:128, :], den[:], ALU.mult)
                        fill_point()
                while emitted < nfill:
                    fillers[emitted]()
                    emitted += 1

            def alltoall(b):
                # shard-major bounce: rows [128j:128(j+1)) = my aoT cols for
                # rank j's 128 tokens of row b; A2A swaps shards so ag_out
                # stacks all ranks' head-dim blocks for MY tokens of row b.
                src = aoT_sb[:, b * T:(b + 1) * T].rearrange(
                    "c (j q) -> c j q", j=NCORES)
                dst = ag_in[b][:].rearrange("(j c) q -> c j q", c=128)
                nc.gpsimd.dma_start(dst, src)
                nc.gpsimd.collective_compute(
                    "AllToAll", ALU.bypass,
                    replica_groups=[list(range(NCORES))],
                    ins=[ag_in[b][:]], outs=[ag_out[b][:]])

            def oproj_units(b):
                """Token-stationary o-proj: y[t, e] for my 128 tokens of row
                b, as two 8-MM chain units (one per 512-wide output half)
                with the PSUM drain + output DMA inside the unit, so the
                first half's copy/DMA overlaps the second half's matmuls."""
                aof = aof_pool.tile([128, CT * 128], BF16, tag="aof",
                                    name=f"aof{b}")
                for ah in range(2):
                    nc.sync.dma_start(
                        aof[:, ah * 512:ah * 512 + 512].rearrange(
                            "c (ct q) -> c ct q", ct=CT // 2),
                        ag_out[b][ah * 512:ah * 512 + 512, :].rearrange(
                            "(ct c) q -> c ct q", c=128))

                def mk(hf, b=b, aof=aof):
                    yp = opsum.tile([128, 512], F32, tag="po",
                                    name=f"yp{b}_{hf}")
                    for ct in range(CT):
                        nc.tensor.matmul(
                            yp[:],
                            aof[:, ct * 128:(ct + 1) * 128],
                            wo_sb[:, ct * D + hf * 512:ct * D + hf * 512 + 512],
                            start=(ct == 0), stop=(ct == CT - 1))
                    yo = yout.tile([128, 512], F32, tag=f"yo{hf}",
                                   name=f"yo{b}_{hf}")
                    if hf:
                        nc.scalar.copy(yo[:], yp[:])
                    else:
                        nc.vector.tensor_copy(yo[:], yp[:])
                    nc.sync.dma_start(
                        yO[b * 128:(b + 1) * 128, hf * 512:hf * 512 + 512],
                        yo[:])
                return [lambda hf=hf: mk(hf) for hf in range(2)]

            # PE warm-up burst: dep-free identity matmuls run back-to-back
            # the moment the identity is built (~9us). The HAM SHORT window
            # needs ~3.4us of SUSTAINED activity to unthrottle; 28 N=128
            # matmuls measured only 3.1us (just under), so use 64 (~7us) to
            # guarantee the gate opens before the row-0 projections start.
            # 40 MMs ~= 4.5us: crosses the 3.4us window but drains before the
            # first x chunk lands, so it never delays the row-0 projections.
            wps = ppool.tile([128, 512], F32, tag="proj", name="warm")
            for i in range(40):
                nc.tensor.matmul(wps[:, 0:128], ident[:], ident[:],
                                 start=True, stop=True)
            # and one matmul chained to each x chunk keeps it open while the
            # rest of row 0 streams in
            for ct in range(CT):
                nc.tensor.matmul(wps[:], ident[:], xrows[0][:, ct, 0:512],
                                 start=True, stop=True)

            for u in qkv_units(0):
                u()
            for b in range(B):
                fillers = []
                if b + 1 < B:
                    fillers.extend(qkv_units(b + 1))
                attention(b, fillers)
                alltoall(b)
                # o-proj of row b-2 strictly AFTER attention(b): a late peer's
                # collective then has ~2.5 rows of slack before its wait could
                # head-of-line-block this core's in-order PE queue (observed
                # inter-core dispatch skew reaches ~35us)
                if b >= 2:
                    for u in oproj_units(b - 2):
                        u()
            for u in oproj_units(2):
                u()
            # paced HAM-keeper chain across the A2A(3) wait: PE<->DVE
            # ping-pong (each matmul's rhs is the junk tile the previous
            # vector copy wrote) advances ~1us per step, marking PE activity
            # through the ~8-20us idle gap so oproj(3) runs at 2.4 GHz
            # instead of cold. 8 steps ~= the collective's best-case floor,
            # so the chain drains before the AllToAll lands and never delays
            # oproj(3).
            kj0 = rtp.tile([128, 128], BF16, tag="sw", name="kj0")
            kj1 = rtp.tile([128, 128], BF16, tag="m1", name="kj1")
            nc.vector.tensor_copy(kj0[:], triu_sb[:])
            kwp = ppool.tile([128, 128], F32, tag="proj", name="kwp")
            for i in range(8):
                nc.tensor.matmul(kwp[:], ident[:], kj0[:] if i % 2 == 0 else kj1[:],
                                 start=True, stop=True)
                nc.vector.tensor_copy(kj1[:] if i % 2 == 0 else kj0[:], kwp[:])
            for u in oproj_units(3):
                u()

    nc.compile()
    return nc


def _host_inputs(x, Wq, Wk, Wv, Wo):
    bf16 = ml_dtypes.bfloat16
    x2 = np.asarray(x, dtype=np.float32).reshape(BT, D)
    xT = np.ascontiguousarray(x2.T).astype(bf16)

    inv_freq = 1.0 / (ROPE_BASE ** (np.arange(0, DH, 2, dtype=np.float32) / DH))
    tpos = np.arange(T, dtype=np.float32)
    freqs = np.outer(tpos, inv_freq).astype(np.float32)   # [T, 32]
    cos = np.cos(freqs).astype(np.float32)
    sin = np.sin(freqs).astype(np.float32)
    pidx = (np.arange(DPC) % DH) // 2
    cosb = np.ascontiguousarray(cos.T[pidx, :]).astype(np.float32)  # [128, T]
    sign = np.where(np.arange(DPC) % 2 == 0, -1.0, 1.0).astype(np.float32)
    sinb = np.ascontiguousarray(sin.T[pidx, :] * sign[:, None]).astype(np.float32)

    triu = np.triu(np.ones((128, 128), np.float32)).astype(bf16)

    def prepack(W, i):
        sl = slice(i * DPC, (i + 1) * DPC)
        wT = np.asarray(W, np.float32)[sl, :].T          # [1024, 128]
        blocks = [wT[ct * 128:(ct + 1) * 128, :] for ct in range(CT)]
        return np.ascontiguousarray(np.concatenate(blocks, axis=1)).astype(bf16)

    # wo packed [128, CT*D]: block ct = Wo.T rows [128ct:128(ct+1)]
    woT = np.ascontiguousarray(np.asarray(Wo, np.float32).T)   # [c, e]
    wo_blocks = [woT[ct * 128:(ct + 1) * 128, :] for ct in range(CT)]
    wo_packed = np.ascontiguousarray(np.concatenate(wo_blocks, axis=1)).astype(bf16)

    in_maps = []
    for i in range(NCORES):
        m = {
            "xT": xT,
            "wq": prepack(Wq, i),
            "wk": prepack(Wk, i),
            "wv": prepack(Wv, i),
            "wo": wo_packed,
            "cosb": cosb.astype(bf16),
            "sinb": sinb.astype(bf16),
            "triu": triu,
        }
        in_maps.append(m)
    return in_maps


def kernel(x, Wq, Wk, Wv, Wo, _trace=False):
    if "nc" not in _compiled:
        _compiled["nc"] = _build_nc()
    nc = _compiled["nc"]
    in_maps = _host_inputs(x, Wq, Wk, Wv, Wo)
    res = run_bass_kernel_spmd(nc, in_maps, list(range(NCORES)), trace=_trace)
    _compiled["last_result"] = res
    # core j holds yO_j [512, 1024]: rows [128b:128(b+1)) = batch row b,
    # tokens [128j:128(j+1)), full 1024 output dims
    y = np.empty((B, T, D), np.float32)
    for j in range(NCORES):
        yo = res.results[j]["yO"]           # [512, 1024]
        for b in range(B):
            y[b, 128 * j:128 * (j + 1), :] = yo[128 * b:128 * (b + 1), :]
    return y


# revision 25
# speedup vs baseline: 1.0409x; 1.0409x over previous
"""Multi-head causal attention (RoPE) on 8 TRN2 NeuronCores.

Sharding: tensor-parallel over heads. Each core computes 2 of the 16 heads:
column-parallel q/k/v projections, local attention, then a per-batch-row
AllToAll of the transposed attention outputs and a token-parallel o-proj
(each core produces the full 1024-wide output for 128 tokens per row).

Layout strategy: activations live transposed on-chip ([dim, token]) so every
matmul contracts over the partition axis with no transposes of x. Scores are
computed transposed ([tk, tq]); softmax has no max-subtraction (logits are
O(1) for this input distribution) and its denominator is produced by a
64-wide ones block appended to V in the PV matmul; normalization is a single
tensor-tensor divide per (b, head, tq-half) writing bf16 aoT directly.
RoPE uses the interleaved-pair identity q' = q*C + swap(q)*S', with the pair
swap done by the DVE stream-shuffle.

o-proj is token-stationary: after the per-row AllToAll each core holds all
1024 attention dims for its 128 tokens of that row; the 128-token tile is the
matmul stationary operand and Wo.T streams as the moving operand (N=512).

Schedule: attention(b) is emitted with next-row QKV projection units and the
o-proj of row b-2 INTERLEAVED between its score/PV groups, so the in-order
PE queue always has ready matmuls behind an exp-gated attention group (keeps
the HAM clock-gate warm and hides dependency stalls). The per-row AllToAll
is issued right after row b's last normalize; o-proj lags two rows so a
late-started peer's collective never head-of-line-blocks the PE queue.
DMA priority: x row 0 (4 chunked DMAs) first, then rows 1-3 (one DMA each),
then Wo on the same FIFO ring so it never competes with the x load.
"""

import sys

for _p in ("/opt/trn_rl_repo",):
    if _p not in sys.path:
        sys.path.insert(0, _p)

import contextlib

import numpy as np
import ml_dtypes

import concourse.bass as bass
import concourse.mybir as mybir
import concourse.tile as tile
from concourse import bacc
from concourse.bass_utils import run_bass_kernel_spmd
from concourse.masks import make_identity

# Problem constants (nn_MultiHeadAttention: x [4,1024,1024], 16 heads)
B, T, D = 4, 1024, 1024
H, DH = 16, 64
NCORES = 8
HPC = H // NCORES          # heads per core = 2
DPC = HPC * DH             # head-dims per core = 128
BT = B * T                 # 4096 tokens
CT = D // 128              # 8 contraction tiles of 128
TPB = T // 128             # 8 key/query 128-tiles per batch row
ROPE_BASE = 10000.0

F32 = mybir.dt.float32
BF16 = mybir.dt.bfloat16
AF = mybir.ActivationFunctionType
ALU = mybir.AluOpType

SWAP_MASK = [i ^ 1 for i in range(32)]  # pair swap within each 32-partition group

_compiled = {}


def _build_nc():
    nc = bacc.Bacc(None, target_bir_lowering=False, debug=False)

    xT = nc.declare_dram_parameter("xT", [D, BT], BF16, isOutput=False)
    # qkv weights prepacked on host to [128, CT*128] (SBUF layout, single DMA)
    wq = nc.declare_dram_parameter("wq", [128, CT * DPC], BF16, isOutput=False)
    wk = nc.declare_dram_parameter("wk", [128, CT * DPC], BF16, isOutput=False)
    wv = nc.declare_dram_parameter("wv", [128, CT * DPC], BF16, isOutput=False)
    # wo packed [128, CT*D]: block ct = Wo.T rows [128ct:128(ct+1)] (all 1024 cols)
    wo = nc.declare_dram_parameter("wo", [128, CT * D], BF16, isOutput=False)
    cosb = nc.declare_dram_parameter("cosb", [DPC, T], BF16, isOutput=False)
    sinb = nc.declare_dram_parameter("sinb", [DPC, T], BF16, isOutput=False)
    triu = nc.declare_dram_parameter("triu", [128, 128], BF16, isOutput=False)
    # output [tokens, e]: rows [128b:128(b+1)] = batch row b, my 128 tokens
    yO = nc.declare_dram_parameter("yO", [B * 128, D], F32, isOutput=True)

    with tile.TileContext(nc) as tc:
        with contextlib.ExitStack() as ctx:
            dram = ctx.enter_context(tc.tile_pool(name="dram", bufs=1, space="DRAM"))
            # per-batch-row AllToAll bounce buffers
            ag_in = [dram.tile([D, 128], BF16, name=f"agin{b}") for b in range(B)]
            ag_out = [dram.tile([D, 128], BF16, name=f"agout{b}") for b in range(B)]

            consts = ctx.enter_context(tc.tile_pool(name="consts", bufs=1))

            # small weights on the scalar (Activation) DMA ring: loaded in
            # parallel with row 0's x chunks on the sync ring
            cos_sb = consts.tile([DPC, T], BF16)
            sin_sb = consts.tile([DPC, T], BF16)
            triu_sb = consts.tile([128, 128], BF16)
            w_sbs = {}
            for wname, w_dr in (("wq", wq), ("wk", wk), ("wv", wv)):
                w_sb = consts.tile(list(w_dr.shape), BF16, name=f"{wname}_sb")
                nc.scalar.dma_start(w_sb[:], w_dr[:])
                w_sbs[wname] = w_sb
            nc.scalar.dma_start(cos_sb[:], cosb[:])
            nc.scalar.dma_start(sin_sb[:], sinb[:])
            nc.scalar.dma_start(triu_sb[:], triu[:])
            wq_sb, wk_sb, wv_sb = (w_sbs[n] for n in ("wq", "wk", "wv"))
            wo_sb = consts.tile([128, CT * D], BF16, name="wo_sb")

            xpool = ctx.enter_context(tc.tile_pool(name="xTp", bufs=1))
            # row 0 in 2 token-half DMAs: a QKV projection group accumulates
            # over ALL 8 ct tiles, so ct-chunked loads stall it mid-group;
            # token-halves instead make the ci=0 groups fully runnable after
            # 1 MB while the ci=1 half streams in. Rows 1-3 as one DMA each
            # (sequencer issue is ~0.7us per dma_start).
            xrows = []
            xrow0 = xpool.tile([128, CT, T], BF16, tag="xr0", name="xrow0")
            for th in range(2):
                tsl = slice(th * 512, th * 512 + 512)
                nc.sync.dma_start(
                    xrow0[:, :, tsl],
                    xT[:, tsl].rearrange("(ct p) t -> p ct t", p=128))
            xrows.append(xrow0)
            for b in range(1, B):
                xrow = xpool.tile([128, CT, T], BF16, tag=f"xr{b}",
                                  name=f"xrow{b}")
                nc.sync.dma_start(
                    xrow[:],
                    xT[:, b * T:(b + 1) * T].rearrange("(ct p) t -> p ct t",
                                                       p=128))
                xrows.append(xrow)
            # wo (2 MB, needed only by o-proj) goes on the sync ring strictly
            # AFTER the x rows: the ring is FIFO, so it never steals x
            # bandwidth during the startup-critical phase
            nc.sync.dma_start(wo_sb[:], wo[:])

            ident = consts.tile([128, 128], BF16)
            make_identity(nc, ident[:])

            pers = ctx.enter_context(tc.tile_pool(name="pers", bufs=1))
            qT_sb = pers.tile([128, BT], BF16)
            kT_sb = pers.tile([128, BT], BF16)
            aoT_sb = pers.tile([128, BT], BF16)
            # persistent [ones | v_h0 | ones | v_h1] PV lhsT tiles; the ones
            # columns are set once and survive across rows
            v_tiles = [pers.tile([128, 256], BF16, name=f"v{kt}")
                       for kt in range(TPB)]
            for kt in range(TPB):
                nc.gpsimd.memset(v_tiles[kt][:, 0:64], 1.0)
                nc.gpsimd.memset(v_tiles[kt][:, 128:192], 1.0)

            ppool = ctx.enter_context(
                tc.tile_pool(name="proj_psum", bufs=2, space="PSUM"))
            rtp = ctx.enter_context(tc.tile_pool(name="rope_tmp", bufs=2))
            vtmp = ctx.enter_context(tc.tile_pool(name="vtmp", bufs=2))
            epool = ctx.enter_context(tc.tile_pool(name="E", bufs=1))
            spsum = ctx.enter_context(
                tc.tile_pool(name="s_psum", bufs=2, space="PSUM"))
            opsum = ctx.enter_context(
                tc.tile_pool(name="o_psum", bufs=2, space="PSUM"))
            aof_pool = ctx.enter_context(tc.tile_pool(name="aof", bufs=2))
            yout = ctx.enter_context(tc.tile_pool(name="yout", bufs=2))

            scale = float(DH) ** -0.5
            vts_store = {b: {} for b in range(B)}

            def qkv_units(b):
                """Per-(chunk, projection) closures: 8-MM groups + RoPE."""
                units = []
                for ci in range(2):
                    ch = 2 * b + ci
                    sl = slice(ch * 512, ch * 512 + 512)
                    tsl = slice(ci * 512, ci * 512 + 512)
                    xsl = slice(ci * 512, ci * 512 + 512)

                    def mk_v(b=b, ci=ci, ch=ch, xsl=xsl):
                        pv = ppool.tile([128, 512], F32, tag="proj",
                                        name=f"pv{ch}")
                        for ct in range(CT):
                            nc.tensor.matmul(
                                pv[:], wv_sb[:, ct * DPC:(ct + 1) * DPC],
                                xrows[b][:, ct, xsl],
                                start=(ct == 0), stop=(ct == CT - 1))
                        vt = vtmp.tile([128, 512], BF16, tag="vt",
                                       name=f"vt{ch}")
                        nc.vector.tensor_copy(vt[:], pv[:])
                        vts_store[b][ci] = vt
                    units.append(mk_v)

                    for wsb, dst, pname in ((wq_sb, qT_sb, "pq"),
                                            (wk_sb, kT_sb, "pk")):
                        def mk_qk(b=b, ch=ch, sl=sl, tsl=tsl, xsl=xsl,
                                  wsb=wsb, dst=dst, pname=pname):
                            pp = ppool.tile([128, 512], F32, tag="proj",
                                            name=f"{pname}{ch}")
                            for ct in range(CT):
                                nc.tensor.matmul(
                                    pp[:], wsb[:, ct * DPC:(ct + 1) * DPC],
                                    xrows[b][:, ct, xsl],
                                    start=(ct == 0), stop=(ct == CT - 1))
                            # stream_shuffle needs an SBUF source; the
                            # cos-mult reads the projection PSUM directly
                            qraw = rtp.tile([128, 512], BF16, tag="qraw",
                                            name=f"qraw{pname}{ch}")
                            nc.scalar.copy(qraw[:], pp[:])
                            sw = rtp.tile([128, 512], BF16, tag="sw",
                                          name=f"sw{pname}{ch}")
                            m1 = rtp.tile([128, 512], BF16, tag="m1",
                                          name=f"m1{pname}{ch}")
                            m2 = rtp.tile([128, 512], BF16, tag="m2",
                                          name=f"m2{pname}{ch}")
                            nc.vector.stream_shuffle(sw[:], qraw[:], SWAP_MASK)
                            nc.vector.tensor_tensor(m1[:], pp[:],
                                                    cos_sb[:, tsl], ALU.mult)
                            nc.vector.tensor_tensor(m2[:], sw[:],
                                                    sin_sb[:, tsl], ALU.mult)
                            nc.vector.tensor_tensor(dst[:, sl], m1[:], m2[:],
                                                    ALU.add)
                        units.append(mk_qk)
                return units

            def transpose_unit(b, kt):
                """[d, t] -> v_tiles[kt] [ones|v_h0|ones|v_h1] columns."""
                vts = vts_store[b]
                pt = ppool.tile([128, 128], BF16, tag="proj", name=f"pt{b}_{kt}")
                nc.tensor.transpose(pt[:], vts[kt // 4][:, (kt % 4) * 128:
                                                        (kt % 4) * 128 + 128],
                                    ident[:])
                nc.scalar.copy(v_tiles[kt][:, 64:128], pt[:, 0:64])
                nc.vector.tensor_copy(v_tiles[kt][:, 192:256], pt[:, 64:128])

            def attention(b, fillers):
                """Scores+exp+PV+normalize for row b; fillers (next-row QKV
                units, o-proj of row b-2) are emitted between groups so the
                PE queue always has ready work behind exp-gated groups."""
                b0 = b * T
                nfill = len(fillers)
                emitted = 0
                point = 0

                def fill_point():
                    nonlocal emitted, point
                    point += 1
                    target = (point * nfill) // 12
                    while emitted < target:
                        fillers[emitted]()
                        emitted += 1

                e_tiles = {}
                for h in range(HPC):
                    for kt in range(TPB):
                        e_tiles[(h, kt)] = epool.tile(
                            [128, T], BF16, tag=f"e{h}_{kt}", name=f"e{b}_{h}_{kt}")
                for kt in range(TPB):
                    transpose_unit(b, kt)
                    lo = kt * 128
                    for h in range(HPC):
                        hsl = slice(h * 64, (h + 1) * 64)
                        ps = spsum.tile([128, T], F32, tag="s", name=f"s{b}_{h}_{kt}")
                        # bank-aligned score matmuls over the valid range only
                        if lo < 512:
                            nc.tensor.matmul(ps[:, lo:512],
                                             kT_sb[hsl, b0 + lo:b0 + lo + 128],
                                             qT_sb[hsl, b0 + lo:b0 + 512],
                                             start=True, stop=True)
                        nc.tensor.matmul(ps[:, max(lo, 512):T],
                                         kT_sb[hsl, b0 + lo:b0 + lo + 128],
                                         qT_sb[hsl, b0 + max(lo, 512):b0 + T],
                                         start=True, stop=True)
                        nc.scalar.activation(e_tiles[(h, kt)][:, lo:T], ps[:, lo:T],
                                             AF.Exp, scale=scale)
                        nc.vector.tensor_tensor(
                            e_tiles[(h, kt)][:, lo:lo + 128],
                            e_tiles[(h, kt)][:, lo:lo + 128],
                            triu_sb[:], ALU.mult)
                    fill_point()
                for h in range(HPC):
                    # lhsT = [ones | v_h]: PSUM rows 0:64 = denom (at base
                    # partition 0, which the custom-DVE reciprocal requires),
                    # rows 64:128 = PV.
                    for half in range(2):
                        c0 = half * 512
                        po = opsum.tile([128, 512], F32, tag="po",
                                        name=f"po{b}_{h}_{half}")
                        nkt = TPB if half else 4
                        for kt in range(nkt):
                            lo = max(kt * 128 - c0, 0)
                            nc.tensor.matmul(
                                po[:, lo:512],
                                v_tiles[kt][:, h * 128:h * 128 + 128],
                                e_tiles[(h, kt)][:, c0 + lo:c0 + 512],
                                start=(kt == 0), stop=(kt == nkt - 1))
                        den = rtp.tile([64, 512], F32, tag="den",
                                      name=f"den{b}_{h}_{half}")
                        nc.vector.reciprocal_approx_fast(den[:], po[0:64, :])
                        nc.vector.tensor_tensor(
                            aoT_sb[h * 64:(h + 1) * 64, b0 + c0:b0 + c0 + 512],
                            po[64:128, :], den[:], ALU.mult)
                        fill_point()
                while emitted < nfill:
                    fillers[emitted]()
                    emitted += 1

            def alltoall(b):
                # shard-major bounce: rows [128j:128(j+1)) = my aoT cols for
                # rank j's 128 tokens of row b; A2A swaps shards so ag_out
                # stacks all ranks' head-dim blocks for MY tokens of row b.
                src = aoT_sb[:, b * T:(b + 1) * T].rearrange(
                    "c (j q) -> c j q", j=NCORES)
                dst = ag_in[b][:].rearrange("(j c) q -> c j q", c=128)
                nc.gpsimd.dma_start(dst, src)
                nc.gpsimd.collective_compute(
                    "AllToAll", ALU.bypass,
                    replica_groups=[list(range(NCORES))],
                    ins=[ag_in[b][:]], outs=[ag_out[b][:]])

            def oproj_units(b):
                """Token-stationary o-proj: y[t, e] for my 128 tokens of row
                b, as two 8-MM chain units (one per 512-wide output half)
                with the PSUM drain + output DMA inside the unit, so the
                first half's copy/DMA overlaps the second half's matmuls."""
                aof = aof_pool.tile([128, CT * 128], BF16, tag="aof",
                                    name=f"aof{b}")
                for ah in range(2):
                    nc.sync.dma_start(
                        aof[:, ah * 512:ah * 512 + 512].rearrange(
                            "c (ct q) -> c ct q", ct=CT // 2),
                        ag_out[b][ah * 512:ah * 512 + 512, :].rearrange(
                            "(ct c) q -> c ct q", c=128))

                def mk(hf, b=b, aof=aof):
                    yp = opsum.tile([128, 512], F32, tag="po",
                                    name=f"yp{b}_{hf}")
                    for ct in range(CT):
                        nc.tensor.matmul(
                            yp[:],
                            aof[:, ct * 128:(ct + 1) * 128],
                            wo_sb[:, ct * D + hf * 512:ct * D + hf * 512 + 512],
                            start=(ct == 0), stop=(ct == CT - 1))
                    yo = yout.tile([128, 512], F32, tag=f"yo{hf}",
                                   name=f"yo{b}_{hf}")
                    if hf:
                        nc.scalar.copy(yo[:], yp[:])
                    else:
                        nc.vector.tensor_copy(yo[:], yp[:])
                    nc.sync.dma_start(
                        yO[b * 128:(b + 1) * 128, hf * 512:hf * 512 + 512],
                        yo[:])
                return [lambda hf=hf: mk(hf) for hf in range(2)]

            # PE warm-up burst: dep-free identity matmuls run back-to-back
            # the moment the identity is built (~9us). The HAM SHORT window
            # needs ~3.4us of SUSTAINED activity to unthrottle; 28 N=128
            # matmuls measured only 3.1us (just under), so use 64 (~7us) to
            # guarantee the gate opens before the row-0 projections start.
            # 40 MMs ~= 4.5us: crosses the 3.4us window but drains before the
            # first x chunk lands, so it never delays the row-0 projections.
            wps = ppool.tile([128, 512], F32, tag="proj", name="warm")
            for i in range(40):
                nc.tensor.matmul(wps[:, 0:128], ident[:], ident[:],
                                 start=True, stop=True)
            # and one matmul chained to each x chunk keeps it open while the
            # rest of row 0 streams in
            for ct in range(CT):
                nc.tensor.matmul(wps[:], ident[:], xrows[0][:, ct, 0:512],
                                 start=True, stop=True)

            for u in qkv_units(0):
                u()
            for b in range(B):
                fillers = []
                if b + 1 < B:
                    fillers.extend(qkv_units(b + 1))
                attention(b, fillers)
                alltoall(b)
                # o-proj of row b-2 strictly AFTER attention(b): a late peer's
                # collective then has ~2.5 rows of slack before its wait could
                # head-of-line-block this core's in-order PE queue (observed
                # inter-core dispatch skew reaches ~35us)
                if b >= 2:
                    for u in oproj_units(b - 2):
                        u()
            for u in oproj_units(2):
                u()
            # paced HAM-keeper chain across the A2A(3) wait: PE<->DVE
            # ping-pong (each matmul's rhs is the junk tile the previous
            # vector copy wrote) advances ~1us per step, marking PE activity
            # through the ~8-20us idle gap so oproj(3) runs at 2.4 GHz
            # instead of cold. 8 steps ~= the collective's best-case floor,
            # so the chain drains before the AllToAll lands and never delays
            # oproj(3).
            kj0 = rtp.tile([128, 128], BF16, tag="sw", name="kj0")
            kj1 = rtp.tile([128, 128], BF16, tag="m1", name="kj1")
            nc.vector.tensor_copy(kj0[:], triu_sb[:])
            kwp = ppool.tile([128, 128], F32, tag="proj", name="kwp")
            for i in range(8):
                nc.tensor.matmul(kwp[:], ident[:], kj0[:] if i % 2 == 0 else kj1[:],
                                 start=True, stop=True)
                nc.vector.tensor_copy(kj1[:] if i % 2 == 0 else kj0[:], kwp[:])
            for u in oproj_units(3):
                u()

    nc.compile()
    return nc


def _host_inputs(x, Wq, Wk, Wv, Wo):
    bf16 = ml_dtypes.bfloat16
    x2 = np.asarray(x, dtype=np.float32).reshape(BT, D)
    xT = np.ascontiguousarray(x2.T).astype(bf16)

    inv_freq = 1.0 / (ROPE_BASE ** (np.arange(0, DH, 2, dtype=np.float32) / DH))
    tpos = np.arange(T, dtype=np.float32)
    freqs = np.outer(tpos, inv_freq).astype(np.float32)   # [T, 32]
    cos = np.cos(freqs).astype(np.float32)
    sin = np.sin(freqs).astype(np.float32)
    pidx = (np.arange(DPC) % DH) // 2
    cosb = np.ascontiguousarray(cos.T[pidx, :]).astype(np.float32)  # [128, T]
    sign = np.where(np.arange(DPC) % 2 == 0, -1.0, 1.0).astype(np.float32)
    sinb = np.ascontiguousarray(sin.T[pidx, :] * sign[:, None]).astype(np.float32)

    triu = np.triu(np.ones((128, 128), np.float32)).astype(bf16)

    def prepack(W, i):
        sl = slice(i * DPC, (i + 1) * DPC)
        wT = np.asarray(W, np.float32)[sl, :].T          # [1024, 128]
        blocks = [wT[ct * 128:(ct + 1) * 128, :] for ct in range(CT)]
        return np.ascontiguousarray(np.concatenate(blocks, axis=1)).astype(bf16)

    # wo packed [128, CT*D]: block ct = Wo.T rows [128ct:128(ct+1)]
    woT = np.ascontiguousarray(np.asarray(Wo, np.float32).T)   # [c, e]
    wo_blocks = [woT[ct * 128:(ct + 1) * 128, :] for ct in range(CT)]
    wo_packed = np.ascontiguousarray(np.concatenate(wo_blocks, axis=1)).astype(bf16)

    in_maps = []
    for i in range(NCORES):
        m = {
            "xT": xT,
            "wq": prepack(Wq, i),
            "wk": prepack(Wk, i),
            "wv": prepack(Wv, i),
            "wo": wo_packed,
            "cosb": cosb.astype(bf16),
            "sinb": sinb.astype(bf16),
            "triu": triu,
        }
        in_maps.append(m)
    return in_maps


def kernel(x, Wq, Wk, Wv, Wo, _trace=False):
    if "nc" not in _compiled:
        _compiled["nc"] = _build_nc()
    nc = _compiled["nc"]
    in_maps = _host_inputs(x, Wq, Wk, Wv, Wo)
    res = run_bass_kernel_spmd(nc, in_maps, list(range(NCORES)), trace=_trace)
    _compiled["last_result"] = res
    # core j holds yO_j [512, 1024]: rows [128b:128(b+1)) = batch row b,
    # tokens [128j:128(j+1)), full 1024 output dims
    y = np.empty((B, T, D), np.float32)
    for j in range(NCORES):
        yo = res.results[j]["yO"]           # [512, 1024]
        for b in range(B):
            y[b, 128 * j:128 * (j + 1), :] = yo[128 * b:128 * (b + 1), :]
    return y
